# revision 8
# baseline (speedup 1.0000x reference)
"""DeepseekV3 top-k router kernel for 8x Trainium2 NeuronCores.

Strategy:
  - Token dim (8192) sharded 8 ways; router weight replicated per core.
  - logits = hidden @ W.T computed as an fp16 hi/lo split matmul (3 PE passes:
    hi*Whi -> psum_hi; (hi*Wlo + lo*Whi) -> psum_lo, lo parts pre-scaled by
    2^11) giving full-fp32-quality logits at the PE's 2-byte streaming rate.
  - The router bias b is folded into psum_hi via a K=2 mini-matmul
    (ones/2^-11 stationary x [b_hi; b_lo] moving), so stage2 starts from
    biased logits directly.
  - stage2 per 128-token tile: one fused (ps_lo*2^-11 + ps_hi) op, sigmoid,
    grouped top-k via per-group max8 + threshold select, weights gathered as
    max8 of threshold-masked raw scores (order-insensitive), normalization
    sum free via accum_out.
  - First matmul group interleaves 3 token tiles so the W-resident load phase
    stays under the DMA bandwidth budget; warmup matmuls on a zero tile lift
    the PE HAM clock gate to 2.4GHz before real work lands.
Host side packs hidden into transposed, fp16-split, per-core tiled layout and
replicates W/biases; device returns top-8 indices (int32) + weights (f32).
"""

import numpy as np

import concourse.bacc as bacc
import concourse.mybir as mybir
from concourse.tile import TileContext
from concourse import bass_utils

H = 7168
E = 256
T = 8192
NCORES = 8
TLOC = T // NCORES          # 1024 tokens per core
MT = 128                    # tokens per tile (PSUM partition dim)
NM = TLOC // MT             # 8 token tiles per core
KT = H // 128               # 56 contraction tiles
W_CH = [1, 1, 2, 4] + [4] * 12   # W resident chunk sizes (k-tiles)
WC = len(W_CH)
W_OFF = [sum(W_CH[:i]) for i in range(WC)]
X0_CH = [1, 1, 2, 4] + [8] * 6   # m=0 x chunk sizes
X_CH = [8] * 7                   # m>0 x chunk sizes
X_OFF = [sum(X_CH[:i]) for i in range(len(X_CH))]
X0_OFF = [sum(X0_CH[:i]) for i in range(len(X0_CH))]
TOP_K = 8
N_GROUP = 8
TOPK_GROUP = 4
EG = E // N_GROUP           # 32 experts per group
SCALE = 2.5
SC = 2048.0                 # 2^11 lo-part pre-scale
NWARM = 20                  # HAM warmup matmuls on zero data

f32 = mybir.dt.float32
f16 = mybir.dt.float16
u32 = mybir.dt.uint32
i32 = mybir.dt.int32
AOT = mybir.AluOpType
ACTF = mybir.ActivationFunctionType

_PROG = None


def _build():
    nc = bacc.Bacc(trn_type="TRN2")
    X = nc.dram_tensor("x", [NM, 128, KT, 2 * MT], f16, kind="ExternalInput")
    Wd = nc.dram_tensor("w", [128, KT, 2 * E], f16, kind="ExternalInput")
    C = nc.dram_tensor("c", [128, E], f32, kind="ExternalInput")
    # rows: [b_hi | ones], [b_lo | 2^-11]  -> K=2 bias matmul operands
    BB = nc.dram_tensor("bb", [2, E + MT], f16, kind="ExternalInput")
    OIDX = nc.dram_tensor("oidx", [TLOC, TOP_K], i32, kind="ExternalOutput")
    OW = nc.dram_tensor("ow", [TLOC, TOP_K], f32, kind="ExternalOutput")

    with TileContext(nc) as tc:
        with (
            tc.tile_pool(name="const", bufs=1) as cpool,
            tc.tile_pool(name="xs", bufs=2) as xpool,
            tc.tile_pool(name="x2", bufs=1) as x2pool,
            tc.tile_pool(name="s2", bufs=2) as s2,
            tc.tile_pool(name="psum", bufs=2, space="PSUM") as pspool,
            tc.tile_pool(name="psum1", bufs=1, space="PSUM") as pspool1,
            tc.tile_pool(name="psum2", bufs=1, space="PSUM") as pspool2,
        ):
            # Tiny consts first on the queues: b rows + warmup zero tile.
            bb_sb = cpool.tile([2, E + MT], f16, name="bb_sb")
            nc.sync.dma_start(bb_sb[:, :], BB[:, :])
            ones2 = bb_sb[:, E:E + MT]
            zz = cpool.tile([128, 2 * MT], f16, name="zz")
            nc.vector.memset(zz[:, :], 0.0)

            def load_x(pool, m, hf, k0, nk, tagp="x"):
                xt = pool.tile([128, nk * 2 * MT], f16, tag=f"{tagp}{hf}",
                               name=f"{tagp}{hf}_{m}")
                nc.sync.dma_start(
                    xt.rearrange("p (k t) -> p k t", k=nk),
                    X[m, :, k0:k0 + nk, :],
                )
                return xt

            # m0,m1,m2 run interleaved so the PE has three tiles of work
            # while the (DMA-heavy) W-resident load streams in; later tiles
            # go single-file.
            groups = [[0, 1, 2]] + [[m] for m in range(3, NM)]

            # Startup DMAs interleaved in the order the k-loop consumes them
            # (the HWDGE queues drain FIFO, so W must not be queued ahead of
            # the first token tiles' activations).
            startup = []
            wi = 0
            xi = {0: 0, 1: 0, 2: 0}
            for k in range(KT):
                while wi < WC and W_OFF[wi] <= k:
                    startup.append(("w", 0, wi)); wi += 1
                while xi[0] < len(X0_CH) and X0_OFF[xi[0]] <= k:
                    startup.append(("x", 0, xi[0])); xi[0] += 1
                for mm in (1, 2):
                    while xi[mm] < len(X_CH) and X_OFF[xi[mm]] <= k:
                        startup.append(("x", mm, xi[mm])); xi[mm] += 1
            w_sbs = [None] * WC
            x_t = {0: [None] * len(X0_CH), 1: [None] * len(X_CH),
                   2: [None] * len(X_CH)}
            for kind, m, i in startup:
                if kind == "w":
                    nk = W_CH[i]
                    wt = cpool.tile([128, nk * 2 * E], f16, name=f"w_sb{i}")
                    nc.sync.dma_start(
                        wt.rearrange("p (k e) -> p k e", k=nk),
                        Wd[:, W_OFF[i]:W_OFF[i] + nk, :],
                    )
                    w_sbs[i] = wt
                elif m == 0:
                    x_t[0][i] = load_x(xpool, 0, i, X0_OFF[i], X0_CH[i])
                elif m == 1:
                    x_t[1][i] = load_x(xpool, 1, i, X_OFF[i], X_CH[i])
                else:
                    x_t[2][i] = load_x(x2pool, 2, i, X_OFF[i], X_CH[i],
                                       tagp="y")

            # eb replicated; needed only by the first stage2 (~45us in).
            c_sb = cpool.tile([128, E], f32, name="c_sb")
            nc.sync.dma_start(c_sb[:, :], C[:, :])
            eb_rep = c_sb[:, 0:E]

            wmap = []
            for ci, n in enumerate(W_CH):
                wmap += [(ci, j) for j in range(n)]

            def xmap_for(ch):
                mp = []
                for ci, n in enumerate(ch):
                    mp += [(ci, j) for j in range(n)]
                return mp

            # HAM warmup: zero matmuls keep the PE busy from the end of the
            # preamble so the 4/8 clock gate lifts before real MMs arrive.
            wps = pspool.tile([128, E], f32, tag="ps_hi0", name="warm_ps")
            for i in range(NWARM):
                nc.tensor.matmul(wps[:, :], zz[:, :128], zz[:, :],
                                 start=(i == 0), stop=(i == NWARM - 1))

            def stage2(m, ps_hi, ps_lo):
                # biased logits: lg = ps_lo/2^11 + ps_hi  (b already in ps_hi)
                hb = s2.tile([128, E], f32, tag="hb", name=f"hb{m}")
                nc.scalar.activation(hb[:, :], ps_lo[:, :], ACTF.Copy,
                                     scale=1.0 / SC)
                lg = s2.tile([128, E], f32, tag="lg", name=f"lg{m}")
                nc.vector.tensor_add(lg[:, :], hb[:, :], ps_hi[:, :])
                s = s2.tile([128, E], f32, tag="s", name=f"s{m}")
                nc.scalar.activation(s[:, :], lg[:, :], ACTF.Sigmoid)
                sfc = s2.tile([128, E], f32, tag="sfc", name=f"sfc{m}")
                nc.vector.tensor_add(sfc[:, :], s[:, :], eb_rep)

                # group scores: sum of top-2 per group of 32
                gmax = s2.tile([128, 8 * N_GROUP], f32, tag="gmax",
                               name=f"gmax{m}")
                for g in range(N_GROUP):
                    nc.vector.max(out=gmax[:, g * 8:(g + 1) * 8],
                                  in_=sfc[:, g * EG:(g + 1) * EG])
                gm3 = gmax.rearrange("p (g c) -> p g c", c=8)
                gs = s2.tile([128, N_GROUP], f32, tag="gs", name=f"gs{m}")
                nc.vector.tensor_add(gs.unsqueeze(2), gm3[:, :, 0:1],
                                     gm3[:, :, 1:2])
                g8 = s2.tile([128, 8], f32, tag="g8", name=f"g8{m}")
                nc.vector.max(out=g8[:, :], in_=gs[:, :])
                gmask = s2.tile([128, N_GROUP], f32, tag="gmask",
                                name=f"gmask{m}")
                nc.vector.tensor_scalar(gmask[:, :], gs[:, :],
                                        g8[:, TOPK_GROUP - 1:TOPK_GROUP],
                                        None, op0=AOT.is_ge)
                # mask sfc to the selected groups, take the 8th value as the
                # selection threshold
                msfc = s2.tile([128, E], f32, tag="msfc", name=f"msfc{m}")
                nc.vector.tensor_mul(
                    msfc.rearrange("p (g c) -> p g c", c=EG),
                    sfc.rearrange("p (g c) -> p g c", c=EG),
                    gmask.unsqueeze(2).to_broadcast([128, N_GROUP, EG]),
                )
                m8 = s2.tile([128, 8], f32, tag="m8", name=f"m8{m}")
                nc.vector.max(out=m8[:, :], in_=msfc[:, :])

                # svals = raw scores of the selected experts (others -> 0);
                # accum_out gives the normalization sum for free.
                svals = s2.tile([128, E], f32, tag="svals", name=f"svals{m}")
                rs = s2.tile([128, 1], f32, tag="rs", name=f"rs{m}")
                nc.vector.scalar_tensor_tensor(
                    svals[:, :], msfc[:, :], m8[:, TOP_K - 1:TOP_K], s[:, :],
                    op0=AOT.is_ge, op1=AOT.mult, accum_out=rs[:, :])
                w8 = s2.tile([128, 8], f32, tag="w8", name=f"w8{m}")
                nc.vector.max(out=w8[:, :], in_=svals[:, :])
                i8 = s2.tile([128, 8], u32, tag="i8", name=f"i8{m}")
                nc.vector.max_index(out=i8[:, :], in_max=w8[:, :],
                                    in_values=svals[:, :])
                nc.sync.dma_start(OIDX[m * MT:(m + 1) * MT, :], i8.bitcast(i32))

                rc = s2.tile([128, 1], f32, tag="rc", name=f"rc{m}")
                nc.vector.reciprocal(rc[:, :], rs[:, :])
                wo = s2.tile([128, 8], f32, tag="wo", name=f"wo{m}")
                nc.vector.tensor_scalar(wo[:, :], w8[:, :], rc[:, 0:1], SCALE,
                                        op0=AOT.mult, op1=AOT.mult)
                nc.sync.dma_start(OW[m * MT:(m + 1) * MT, :], wo[:, :])

            for group in groups:
                pss = {}
                for gi, m in enumerate(group):
                    pool = (pspool, pspool1, pspool2)[gi]
                    pss[m] = (
                        pool.tile([128, E], f32, tag=f"ps_hi{gi}",
                                  name=f"ps_hi{m}"),
                        pool.tile([128, E], f32, tag=f"ps_lo{gi}",
                                  name=f"ps_lo{m}"),
                    )
                xts = {}
                for m in group:
                    if m in x_t:
                        xts[m] = (x_t[m],
                                  xmap_for(X0_CH if m == 0 else X_CH))
                    else:
                        xts[m] = ([load_x(xpool, m, hf, X_OFF[hf], X_CH[hf])
                                   for hf in range(len(X_CH))],
                                  xmap_for(X_CH))
                # fold b into ps_hi: 1*b_hi + 2^-11*b_lo
                for m in group:
                    nc.tensor.matmul(pss[m][0][:, :], ones2, bb_sb[:, 0:E],
                                     start=True, stop=False)
                for k in range(KT):
                    wc, kw = wmap[k]
                    wt = w_sbs[wc]
                    wh = wt[:, kw * 2 * E: kw * 2 * E + E]
                    wl = wt[:, kw * 2 * E + E: (kw + 1) * 2 * E]
                    for m in group:
                        xtl, xmp = xts[m]
                        xc, kl = xmp[k]
                        xt = xtl[xc]
                        xh = xt[:, kl * 2 * MT: kl * 2 * MT + MT]
                        xl = xt[:, kl * 2 * MT + MT: (kl + 1) * 2 * MT]
                        ps_hi, ps_lo = pss[m]
                        # xh stationary twice in a row -> one LDWEIGHTS shared
                        nc.tensor.matmul(ps_hi[:, :], xh, wh,
                                         start=False, stop=(k == KT - 1))
                        nc.tensor.matmul(ps_lo[:, :], xh, wl,
                                         start=(k == 0), stop=False)
                        nc.tensor.matmul(ps_lo[:, :], xl, wh,
                                         start=False, stop=(k == KT - 1))
                for m in group:
                    stage2(m, *pss[m])

    nc.finalize()
    return nc


def _pack_hidden(x_shard: np.ndarray) -> np.ndarray:
    """[TLOC, H] f32 -> [NM, 128, KT, 2*MT] f16 (hi | scaled lo per k-tile)."""
    xT = np.ascontiguousarray(x_shard.T)               # [H, TLOC]
    xh = xT.astype(np.float16)
    xl = ((xT - xh.astype(np.float32)) * SC).astype(np.float16)
    out = np.empty((NM, 128, KT, 2 * MT), np.float16)
    # xh[k*128+p, m*128+t] -> out[m, p, k, t]
    xh4 = xh.reshape(KT, 128, NM, MT).transpose(2, 1, 0, 3)
    xl4 = xl.reshape(KT, 128, NM, MT).transpose(2, 1, 0, 3)
    out[:, :, :, :MT] = xh4
    out[:, :, :, MT:] = xl4
    return np.ascontiguousarray(out)


def _pack_w(W: np.ndarray) -> np.ndarray:
    """[E, H] f32 -> [128, KT, 2*E] f16 (hi | scaled lo)."""
    wT = np.ascontiguousarray(W.T)                     # [H, E]
    wh = wT.astype(np.float16)
    wl = ((wT - wh.astype(np.float32)) * SC).astype(np.float16)
    out = np.empty((128, KT, 2 * E), np.float16)
    out[:, :, :E] = wh.reshape(KT, 128, E).transpose(1, 0, 2)
    out[:, :, E:] = wl.reshape(KT, 128, E).transpose(1, 0, 2)
    return np.ascontiguousarray(out)


def _make_consts(b: np.ndarray, eb: np.ndarray):
    c = np.ascontiguousarray(np.broadcast_to(eb[None, :], (128, E)),
                             dtype=np.float32)
    bh = b.astype(np.float16)
    bl = ((b - bh.astype(np.float32)) * SC).astype(np.float16)
    bb = np.empty((2, E + MT), np.float16)
    bb[0, :E] = bh
    bb[1, :E] = bl
    bb[0, E:] = 1.0
    bb[1, E:] = np.float16(2.0 ** -11)
    return c, bb


def kernel(hidden_states, W, b, e_score_correction_bias):
    global _PROG
    hidden_states = np.asarray(hidden_states, np.float32)
    W = np.asarray(W, np.float32)
    b = np.asarray(b, np.float32)
    eb = np.asarray(e_score_correction_bias, np.float32)

    if _PROG is None:
        _PROG = _build()
    nc = _PROG

    wp = _pack_w(W)
    c, bb = _make_consts(b, eb)

    in_maps = []
    for cid in range(NCORES):
        shard = hidden_states[cid * TLOC:(cid + 1) * TLOC]
        in_maps.append({"x": _pack_hidden(shard), "w": wp, "c": c, "bb": bb})

    res = bass_utils.run_bass_kernel_spmd(nc, in_maps,
                                          core_ids=list(range(NCORES)))

    idx = np.concatenate([res.results[cid]["oidx"] for cid in range(NCORES)],
                         axis=0)
    wts = np.concatenate([res.results[cid]["ow"] for cid in range(NCORES)],
                         axis=0)
    return idx.astype(np.int32), wts.astype(np.float32)
